# revision 2
# baseline (speedup 1.0000x reference)
"""Trainium2 Bass kernel for nn_AttentionLSTM_13529146983094.

The reference network ends with ``layer_norm`` over a size-1 feature dim:
``m = mean(h, -1) == h`` exactly, so ``(h - m) == 0`` exactly in fp32 and the
normalized value collapses to the bias: ``h[b,s] = ln2_b[0]`` for every
element.  The final projection therefore reduces, exactly (not approximately),
to

    out[b, o] = ln2_b[0] * sum_s Wf[o, s] + bf[o]

independent of ``x`` and every LSTM/attention parameter.  The kernel computes
that 30-vector on device and broadcasts it to the [1024, 30] output shard on
each of the 8 cores (batch 8192 sharded data-parallel, per the hint).
"""

import numpy as np

import concourse.mybir as mybir
from concourse import bacc
from concourse.tile import TileContext
from concourse.bass_utils import run_bass_kernel_spmd

N_CORES = 8
B = 8192
BS = B // N_CORES          # 1024 batch rows per core
OUT_LEN = 30
SEQ = 90
P = 128                    # SBUF partitions
RPP = BS // P              # 8 output rows per partition
F32 = mybir.dt.float32


def _build_nc():
    nc = bacc.Bacc(None)
    Wf = nc.declare_dram_parameter("Wf", [OUT_LEN, SEQ], F32, isOutput=False)
    bf = nc.declare_dram_parameter("bf", [OUT_LEN], F32, isOutput=False)
    ln2_b = nc.declare_dram_parameter("ln2_b", [1], F32, isOutput=False)
    out = nc.declare_dram_parameter("out", [BS, OUT_LEN], F32, isOutput=True)

    with TileContext(nc) as tc:
        with (
            tc.tile_pool(name="sbuf", bufs=1) as pool,
            tc.tile_pool(name="psum", bufs=1, space="PSUM") as pp,
        ):
            # Wf transposed into SBUF: [SEQ partitions, OUT_LEN free]
            wft = pool.tile([SEQ, OUT_LEN], F32)
            nc.sync.dma_start(out=wft[:], in_=Wf[:, :].transpose([1, 0]))
            bfr = pool.tile([1, OUT_LEN], F32)
            nc.sync.dma_start(out=bfr[:], in_=bf[:].unsqueeze(0))
            lnb = pool.tile([1, 1], F32)
            nc.sync.dma_start(out=lnb[:], in_=ln2_b[:].unsqueeze(0))

            # sums[o] = sum_s Wf[o, s]  via ones(SEQ).T @ WfT  -> [1, OUT_LEN]
            ones_seq = pool.tile([SEQ, 1], F32)
            nc.vector.memset(ones_seq[:], 1.0)
            sums_p = pp.tile([1, OUT_LEN], F32)
            nc.tensor.matmul(sums_p[:], lhsT=ones_seq[:], rhs=wft[:], start=True, stop=True)

            # v = ln2_b * sums + bf   [1, OUT_LEN]
            v = pool.tile([1, OUT_LEN], F32)
            nc.vector.tensor_scalar_mul(v[:], sums_p[:], lnb[:])
            nc.vector.tensor_add(out=v[:], in0=v[:], in1=bfr[:])

            # v replicated RPP times along free dim: [1, RPP*OUT_LEN]
            v8 = pool.tile([1, RPP * OUT_LEN], F32)
            nc.vector.tensor_copy(
                out=v8[:].rearrange("p (r o) -> p r o", o=OUT_LEN),
                in_=v[:].unsqueeze(1).broadcast_to([1, RPP, OUT_LEN]),
            )

            # outer product ones(P) x v8 broadcasts to all partitions
            ones_col = pool.tile([1, P], F32)
            nc.vector.memset(ones_col[:], 1.0)
            big_p = pp.tile([P, RPP * OUT_LEN], F32)
            nc.tensor.matmul(big_p[:], lhsT=ones_col[:], rhs=v8[:], start=True, stop=True)
            big = pool.tile([P, RPP * OUT_LEN], F32)
            nc.vector.tensor_copy(out=big[:], in_=big_p[:])

            nc.sync.dma_start(
                out=out[:, :].rearrange("(p r) o -> p (r o)", p=P),
                in_=big[:],
            )
    nc.compile()
    return nc


def _run(inputs, trace=False, **kw):
    in_map = {
        k: np.ascontiguousarray(np.asarray(inputs[k], dtype=np.float32))
        for k in ("Wf", "bf", "ln2_b")
    }
    nc = _build_nc()
    res = run_bass_kernel_spmd(
        nc, [in_map] * N_CORES, core_ids=list(range(N_CORES)), trace=trace, **kw
    )
    full = np.concatenate(
        [np.asarray(res.results[i]["out"]) for i in range(N_CORES)], axis=0
    )
    return full, res


def kernel(**inputs):
    full, _ = _run(inputs)
    return full


# revision 3
# speedup vs baseline: 1.3176x; 1.3176x over previous
"""v4 = v3 math + contiguous out-DMA (replication in the DVE copy) + preamble
hoist: the kernel's 8 instructions are moved to the FRONT of the entry block so
the framework preamble (~7.5us of engine init) runs concurrently with the
DMA -> PE -> DVE -> DMA chain instead of before it.

out[b, o] = ln2_b[0] * sum_s Wf[o, s] + bf[o]   (exact; see kernel.py)

packed[91, 31]: cols 0..29 = vstack(Wf.T, bf); col 30 = [ln2_b]*90 + [1.0].
  v128[128,30](PSUM) = packed[:,30:31].bcast([91,128]).T @ packed[:,0:30]
  big[128,240](SBUF) = v128 replicated 8x along free (DVE bcast read)
  out[1024,30]       = big  (one contiguous 123KB DMA, 128 x 960B)
"""

import numpy as np

import concourse.bass as bass
import concourse.mybir as mybir
from concourse.bass_utils import run_bass_kernel_spmd

N_CORES = 8
B = 8192
BS = B // N_CORES
OUT_LEN = 30
SEQ = 90
P = 128
RPP = BS // P  # 8
K = SEQ + 1    # 91
F32 = mybir.dt.float32

HOIST = True


def _build_nc(hoist=None):
    hoist = HOIST if hoist is None else hoist
    nc = bass.Bass(enable_partition_id=False, monotonic_sem_count=0)
    packed = nc.declare_dram_parameter("packed", [K, OUT_LEN + 1], F32, isOutput=False)
    out = nc.declare_dram_parameter("out", [BS, OUT_LEN], F32, isOutput=True)

    with (
        nc.sbuf_tensor([K, OUT_LEN + 1], F32) as pk,
        nc.sbuf_tensor([P, RPP * OUT_LEN], F32) as big,
        nc.psum_tensor([P, OUT_LEN], F32) as big_p,
        nc.semaphore("dsem") as dsem,
        nc.semaphore("psem") as psem,
        nc.semaphore("vsem") as vsem,
        nc.semaphore("osem") as osem,
        nc.Block() as block,
    ):

        @block.sync
        def _(sync: bass.BassEngine):
            sync.dma_start(out=pk[:], in_=packed[:, :]).then_inc(dsem, 16)
            sync.wait_ge(vsem, 1)
            sync.dma_start(
                out=out[:, :].rearrange("(p r) o -> p (r o)", p=P), in_=big[:]
            ).then_inc(osem, 16)
            sync.wait_ge(osem, 16)

        @block.tensor
        def _(tensor: bass.BassEngine):
            tensor.wait_ge(dsem, 16)
            tensor.matmul(
                big_p[:],
                lhsT=pk[:, OUT_LEN : OUT_LEN + 1].broadcast_to([K, P]),
                rhs=pk[:, 0:OUT_LEN],
                start=True,
                stop=True,
            ).then_inc(psem, 1)

        @block.vector
        def _(vector: bass.BassEngine):
            vector.wait_ge(psem, 1)
            vector.tensor_copy(
                out=big[:].rearrange("p (r o) -> p r o", o=OUT_LEN),
                in_=big_p[:].unsqueeze(1).broadcast_to([P, RPP, OUT_LEN]),
            ).then_inc(vsem, 1)

    if hoist:
        _hoist_user_instructions(nc)
    return nc


def _hoist_user_instructions(nc):
    """Move the kernel's instructions from the per-engine tail blocks to the
    front of the entry block, so they execute before (concurrently with) the
    framework preamble instead of after it.  Per-engine program order is
    preserved; the semaphore graph is position-independent."""
    my_sems = {"dsem", "psem", "vsem", "osem"}

    def is_mine(ins):
        si = ins.sync_info
        if si is None:
            return False
        names = {w.ant_name for w in si.on_wait} | {u.ant_name for u in si.on_update}
        return bool(names & my_sems)

    blocks = nc.main_func.blocks
    b0 = blocks[0]
    moved = []
    for bb in blocks[1:]:
        if any(type(ins).__name__ == "InstDrain" for ins in bb.instructions):
            continue  # teardown block — leave untouched
        keep = []
        for ins in bb.instructions:
            if is_mine(ins):
                moved.append(ins)
            else:
                keep.append(ins)
        bb.instructions[:] = keep
    assert len(moved) == 8, f"expected to hoist 8 instructions, got {len(moved)}"
    b0.instructions[0:0] = moved


def _pack(inputs):
    Wf = np.asarray(inputs["Wf"], dtype=np.float32)
    bf = np.asarray(inputs["bf"], dtype=np.float32)
    lnb = np.asarray(inputs["ln2_b"], dtype=np.float32)
    packed = np.empty((K, OUT_LEN + 1), dtype=np.float32)
    packed[:SEQ, :OUT_LEN] = Wf.T
    packed[SEQ, :OUT_LEN] = bf
    packed[:SEQ, OUT_LEN] = lnb[0]
    packed[SEQ, OUT_LEN] = 1.0
    return np.ascontiguousarray(packed)


def _run(inputs, trace=False, **kw):
    in_map = {"packed": _pack(inputs)}
    nc = _build_nc()
    res = run_bass_kernel_spmd(
        nc, [in_map] * N_CORES, core_ids=list(range(N_CORES)), trace=trace, **kw
    )
    full = np.concatenate(
        [np.asarray(res.results[i]["out"]) for i in range(N_CORES)], axis=0
    )
    return full, res


def kernel(**inputs):
    full, _ = _run(inputs)
    return full
